# revision 1
# baseline (speedup 1.0000x reference)
"""MultiHeadAttention Trainium2 kernel (8 NeuronCores).

Sharding: batch (2) x head-groups (4): core c -> batch c//4, heads [4*(c%4), 4*(c%4)+4).
Per core: PE-transpose inputs (f32), project q/k/v in bf16 (bias folded), attention in
transposed-score orientation (exp on ACT with the 1/sqrt(d) scale folded, softmax
denominator via a fused ones-column in V), per-seq-window output projection of this
core's head block, summed across the 4-core batch group with ReduceScatter.
Core (b,g) emits global rows {512*c + 128*g + i} of batch b as out[128c+i].
"""

import sys

if "/opt/trn_rl_repo" not in sys.path:
    sys.path.insert(0, "/opt/trn_rl_repo")

import numpy as np

import concourse.bass as bass
import concourse.tile as tile
from concourse import bacc, mybir
from concourse.bass_utils import run_bass_kernel_spmd

B, S, D, H, HD = 2, 2048, 1024, 16, 64
NCORES, GROUP = 8, 4          # 4 cores per batch
HPC = 4                       # heads per core
DPC = HPC * HD                # 256 head-dims per core
SCALE = float(HD) ** -0.5

f32 = mybir.dt.float32
bf16 = mybir.dt.bfloat16
Act = mybir.ActivationFunctionType


def build(seq=S, collective=True, repeat=1):
    """Build the SPMD module (identical program on all 8 cores).

    collective=False replaces the ReduceScatter with a direct copy of this
    core's partial (numerically wrong, for TimelineSim profiling only)."""
    nc = bacc.Bacc("TRN2", target_bir_lowering=False, debug=False,
                   num_devices=NCORES)
    ST = seq // 128           # seq tiles of 128
    NCHUNK = seq // 512       # sq chunks of 512

    # ---- DRAM I/O (per-core shapes) ----
    xq = nc.dram_tensor("xq", [seq, D], f32, kind="ExternalInput").ap()
    xk = nc.dram_tensor("xk", [seq, D], f32, kind="ExternalInput").ap()
    xv = nc.dram_tensor("xv", [seq, D], f32, kind="ExternalInput").ap()
    wqT = nc.dram_tensor("wqT", [D, DPC], f32, kind="ExternalInput").ap()
    wkT = nc.dram_tensor("wkT", [D, DPC], f32, kind="ExternalInput").ap()
    wvT = nc.dram_tensor("wvT", [D, DPC], f32, kind="ExternalInput").ap()
    woT = nc.dram_tensor("woT", [DPC, D], f32, kind="ExternalInput").ap()
    bq = nc.dram_tensor("bq", [DPC, 1], f32, kind="ExternalInput").ap()
    bk = nc.dram_tensor("bk", [DPC, 1], f32, kind="ExternalInput").ap()
    bv = nc.dram_tensor("bv", [1, DPC], f32, kind="ExternalInput").ap()
    bo = nc.dram_tensor("bo", [1, D], f32, kind="ExternalInput").ap()
    ident = nc.dram_tensor("ident", [128, 128], f32, kind="ExternalInput").ap()
    out = nc.dram_tensor("out", [128 * NCHUNK, D], f32, kind="ExternalOutput").ap()

    with tile.TileContext(nc) as tc:
        with (
            tc.tile_pool(name="sb", bufs=2) as sb,
            tc.tile_pool(name="ps", bufs=2, space="PSUM") as psp,
            tc.tile_pool(name="dram", bufs=1, space="DRAM") as dramp,
        ):
            # constants
            id_sb = sb.tile([128, 128], f32, tag="const", bufs=1, name="id_sb")
            nc.sync.dma_start(id_sb[:], ident[:])
            id_bf = sb.tile([128, 128], bf16, tag="const_bf", bufs=1, name="id_bf")
            nc.vector.tensor_copy(id_bf[:], id_sb[:])
            ones_bf = sb.tile([1, 128], bf16, tag="ones", bufs=1, name="ones_bf")
            nc.vector.memset(ones_bf[:], 1.0)
            quarter_bf = sb.tile([1, 128], bf16, tag="quarter", bufs=1,
                                 name="quarter_bf")
            nc.vector.memset(quarter_bf[:], 1.0 / GROUP)
            stage_bv = sb.tile([1, DPC], f32, tag="bvf", bufs=1, name="stage_bv")
            nc.sync.dma_start(stage_bv[:], bv[:])
            bv_bf = sb.tile([1, DPC], bf16, tag="bvb", bufs=1, name="bv_bf")
            nc.vector.tensor_copy(bv_bf[:], stage_bv[:])
            stage_bo = sb.tile([1, D], f32, tag="bof", bufs=1, name="stage_bo")
            nc.sync.dma_start(stage_bo[:], bo[:])
            bo_bf = sb.tile([1, D], bf16, tag="bob", bufs=1, name="bo_bf")
            nc.vector.tensor_copy(bo_bf[:], stage_bo[:])
            bq_sb = [sb.tile([128, 1], f32, tag="bias", bufs=4, name=f"bq_sb{m}")
                     for m in range(2)]
            bk_sb = [sb.tile([128, 1], f32, tag="bias", bufs=4, name=f"bk_sb{m}")
                     for m in range(2)]
            for m in range(2):
                nc.sync.dma_start(bq_sb[m][:], bq[m * 128:(m + 1) * 128, :])
                nc.sync.dma_start(bk_sb[m][:], bk[m * 128:(m + 1) * 128, :])

            def transpose_window(x_ap, w, on_dve=False):
                """Load x rows [512w, 512w+512), cast bf16, transpose.

                Returns 8 tiles [128, 512] bf16: tile k = x[512w:512w+512, 128k:..].T"""
                raws = []
                for j in range(4):
                    r = sb.tile([128, D], f32, tag="xraw", bufs=10, name="xraw_t")
                    st = w * 4 + j
                    nc.sync.dma_start(r[:], x_ap[st * 128:(st + 1) * 128, :])
                    rb = sb.tile([128, D], bf16, tag="xrawb", bufs=10, name="xrawb_t")
                    nc.gpsimd.tensor_copy(rb[:], r[:])
                    raws.append(rb)
                xTw = []
                for dt8 in range(8):
                    ps = psp.tile([128, 512], f32, tag="misc", bufs=2, name="tp_ps")
                    psv = ps[:].bitcast(bf16)
                    for j in range(4):
                        nc.tensor.matmul(
                            psv[:, j * 128:(j + 1) * 128],
                            raws[j][:, dt8 * 128:(dt8 + 1) * 128],
                            id_bf[:],
                            is_transpose=True,
                            start=(j == 0), stop=(j == 3),
                        )
                    xt = sb.tile([128, 512], bf16, tag="xT", bufs=16, name="xTw_t")
                    if on_dve:
                        nc.vector.tensor_copy(xt[:], psv[:, :512])
                    else:
                        nc.scalar.copy(xt[:], psv[:, :512])
                    xTw.append(xt)
                return xTw

            def load_w(w_ap, nm):
                """Weight [D, DPC] f32 -> 8 bf16 tiles [128, DPC]."""
                w_bf = []
                for k in range(8):
                    stg = sb.tile([128, DPC], f32, tag="wstg", bufs=4, name="wstg")
                    nc.sync.dma_start(stg[:], w_ap[k * 128:(k + 1) * 128, :])
                    wb = sb.tile([128, DPC], bf16, tag="w", bufs=24, name=f"{nm}{k}")
                    nc.vector.tensor_copy(wb[:], stg[:])
                    w_bf.append(wb)
                return w_bf

            def proj_T(xTw, w_bf, b_sb, out_tiles, c, on_dve=False):
                """out_tiles[m][:, 512c..] (bf16) = (x-window @ W.T).T + bias."""
                for m in range(2):
                    if True:
                        ps = psp.tile([128, 512], f32, tag="misc", bufs=2, name="pj_ps")
                        for k in range(8):
                            nc.tensor.matmul(
                                ps[:, :512],
                                w_bf[k][:, m * 128:(m + 1) * 128],
                                xTw[k][:, :512],
                                start=(k == 0), stop=(k == 7),
                            )
                        if on_dve:
                            nc.vector.tensor_scalar_add(
                                out_tiles[m][:, c * 512:(c + 1) * 512], ps[:, :512],
                                b_sb[m][:, 0:1])
                        else:
                            nc.scalar.activation(
                                out_tiles[m][:, c * 512:(c + 1) * 512], ps[:, :512],
                                Act.Identity, bias=b_sb[m][:, 0:1], scale=1.0)

            def proj_V_window(xTw, w_bf, w, v_aug):
                """v_aug tiles for stiles 4w..4w+3: head h cols [65h,65h+64)=v, 65h+64=1."""
                for j in range(4):
                    st = 4 * w + j
                    ps = psp.tile([128, 512], f32, tag="misc", bufs=2, name="pv_ps")
                    for k in range(8):
                        nc.tensor.matmul(
                            ps[:, :DPC],
                            xTw[k][:, j * 128:(j + 1) * 128],
                            w_bf[k][:, :],
                            start=(k == 0), stop=False,
                        )
                    nc.tensor.matmul(
                        ps[:, :DPC],
                        ones_bf[0:1, :],
                        bv_bf[0:1, :],
                        start=False, stop=True,
                    )
                    va = sb.tile([128, HPC * 65], bf16, tag="vaug", bufs=ST,
                                 name=f"vaug{st}")
                    nc.gpsimd.memset(va[:], 1.0)
                    for h in range(HPC):
                        nc.vector.tensor_copy(
                            va[:, 65 * h:65 * h + 64],
                            ps[:, 64 * h:64 * h + 64])
                    v_aug.append(va)

            for _rep in range(repeat):
                # ---- persistent tiles ----
                qT = [sb.tile([128, seq], bf16, tag="qkT", bufs=4, name=f"qT{m}")
                      for m in range(2)]
                kT = [sb.tile([128, seq], bf16, tag="qkT", bufs=4, name=f"kT{m}")
                      for m in range(2)]
                OT = [sb.tile([128, seq], bf16, tag="OT", bufs=2, name=f"OT{m}")
                      for m in range(2)]
                wo_bf = []
                for pair in range(2):
                    stg = sb.tile([128, D], f32, tag="xraw", bufs=10, name="wo_stage")
                    nc.sync.dma_start(stg[:], woT[pair * 128:(pair + 1) * 128, :])
                    wb = sb.tile([128, D], bf16, tag="wobf", bufs=2, name=f"wobf{pair}")
                    nc.vector.tensor_copy(wb[:], stg[:])
                    wo_bf.append(wb)
                groups = [[0, 1, 2, 3], [4, 5, 6, 7]]

                def outproj_t(c, t, rs_in):
                    sq = c * 4 + t
                    y_sb = sb.tile([128, D], bf16, tag="y", bufs=6, name="y_sb")
                    for oc in range(2):
                        ps = psp.tile([128, 512], f32, tag="misc", bufs=2,
                                      name="yo_ps")
                        for pair in range(2):
                            nc.tensor.matmul(
                                ps[:, :512],
                                OT[pair][:, sq * 128:(sq + 1) * 128],
                                wo_bf[pair][:, oc * 512:(oc + 1) * 512],
                                start=(pair == 0), stop=False,
                            )
                        nc.tensor.matmul(
                            ps[:, :512],
                            quarter_bf[0:1, :],
                            bo_bf[0:1, oc * 512:(oc + 1) * 512],
                            start=False, stop=True,
                        )
                        nc.vector.tensor_copy(y_sb[:, oc * 512:(oc + 1) * 512],
                                              ps[:, :512])
                    nc.sync.dma_start(rs_in[t * 128:(t + 1) * 128, :], y_sb[:])

                def new_rs_in():
                    return dramp.tile([512, D], bf16, tag="rs_in", bufs=2, name="rs_in")

                def rs_finish(c, rs_in):
                    rs_out = dramp.tile([128, D], bf16, tag="rs_out", bufs=2,
                                        name="rs_out")
                    if collective:
                        nc.gpsimd.collective_compute(
                            "ReduceScatter", mybir.AluOpType.add,
                            replica_groups=groups,
                            ins=[rs_in[:].opt()],
                            outs=[rs_out[:].opt()],
                        )
                    else:
                        nc.sync.dma_start(rs_out[:], rs_in[0:128, :])
                    yb = sb.tile([128, D], bf16, tag="yb", bufs=2, name="yb")
                    nc.sync.dma_start(yb[:], rs_out[:])
                    yf = sb.tile([128, D], f32, tag="yf", bufs=2, name="yf")
                    nc.vector.tensor_copy(yf[:], yb[:])
                    nc.sync.dma_start(out[c * 128:(c + 1) * 128, :], yf[:])

                def outproj_rs(c):
                    rs_in = new_rs_in()
                    for t in range(4):
                        outproj_t(c, t, rs_in)
                    rs_finish(c, rs_in)

                def attn_kt(c, pair, kt, acc):
                    stp = psp.tile([128, 1024], f32, tag="st", bufs=2, name="stp")
                    for hh in range(2):
                        nc.tensor.matmul(
                            stp[:, hh * 512:(hh + 1) * 512],
                            kT[pair][64 * hh:64 * hh + 64, kt * 128:(kt + 1) * 128],
                            qT[pair][64 * hh:64 * hh + 64, c * 512:(c + 1) * 512],
                            start=True, stop=True,
                        )
                    E = sb.tile([128, 1024], bf16, tag="E", bufs=5, name="E_t")
                    nc.scalar.activation(E[:], stp[:], Act.Exp, scale=SCALE)
                    for hh in range(2):
                        h = 2 * pair + hh
                        for t in range(4):
                            nc.tensor.matmul(
                                acc[hh][:, 65 * t:65 * t + 65],
                                E[:, hh * 512 + t * 128:hh * 512 + (t + 1) * 128],
                                v_aug[kt][:, 65 * h:65 * h + 65],
                                start=(kt == 0 and t == 0),
                                stop=(kt == ST - 1 and t == 3),
                            )

                def normalize(c, pair, acc):
                    for hh in range(2):
                        for t in range(4):
                            rc = sb.tile([128, 1], f32, tag="rc", bufs=8, name="rc_t")
                            nc.vector.reciprocal(
                                rc[:], acc[hh][:, 65 * t + 64:65 * t + 65])
                            o_sb = sb.tile([128, 64], bf16, tag="o", bufs=8, name="o_t")
                            nc.vector.tensor_scalar_mul(
                                o_sb[:], acc[hh][:, 65 * t:65 * t + 64], rc[:, 0:1])
                            otp = psp.tile([128, 512], f32, tag="misc", bufs=2,
                                           name="otp_ps")
                            otpv = otp[:].bitcast(bf16)
                            nc.tensor.matmul(
                                otpv[0:64, 0:128],
                                o_sb[:],
                                id_bf[:],
                                is_transpose=True,
                                start=True, stop=True,
                            )
                            sq = c * 4 + t
                            nc.vector.tensor_copy(
                                OT[pair][64 * hh:64 * hh + 64,
                                         sq * 128:(sq + 1) * 128],
                                otpv[0:64, 0:128])

                # ---- fused phase 0 + attention(c0, pair0) per window ----
                v_aug = []
                xTw = transpose_window(xq, 0, on_dve=True)
                wq_bf = load_w(wqT, "wq")
                proj_T(xTw, wq_bf, bq_sb, qT, 0, on_dve=True)
                wk_bf = load_w(wkT, "wk")
                wv_bf = load_w(wvT, "wv")
                acc00 = [psp.tile([128, 4 * 65], f32, tag="acc", bufs=2,
                                  name=f"acc00_{hh}") for hh in range(2)]
                for w in range(NCHUNK):
                    xTw = transpose_window(xk, w, on_dve=True)
                    proj_T(xTw, wk_bf, bk_sb, kT, w)
                    xTw = transpose_window(xv, w)
                    proj_V_window(xTw, wv_bf, w, v_aug)
                    for kt in range(4 * w, 4 * w + 4):
                        attn_kt(0, 0, kt, acc00)
                pending = [(0, 0, acc00)]

                # ---- remaining (c, pair) attention + pipelined qT/outproj/RS ----
                for c in range(NCHUNK):
                    for pair in range(2):
                        if c == 0 and pair == 0:
                            continue
                        acc = [psp.tile([128, 4 * 65], f32, tag="acc", bufs=2,
                                        name=f"acc{hh}") for hh in range(2)]
                        rs_in = None
                        for kt in range(ST):
                            attn_kt(c, pair, kt, acc)
                            if kt == 0 and pending:
                                normalize(*pending.pop())
                            if pair == 1 and c > 0:
                                if kt == 1:
                                    rs_in = new_rs_in()
                                if 1 <= kt <= 4:
                                    outproj_t(c - 1, kt - 1, rs_in)
                                elif kt == 5:
                                    rs_finish(c - 1, rs_in)
                            if pair == 1 and kt == 7 and c + 1 < NCHUNK:
                                xTw = transpose_window(xq, c + 1, on_dve=True)
                                proj_T(xTw, wq_bf, bq_sb, qT, c + 1, on_dve=True)
                        pending.append((c, pair, acc))
                while pending:
                    normalize(*pending.pop())
                outproj_rs(NCHUNK - 1)

    nc.compile()
    return nc


def make_in_maps(query, key, value, Wq, bq_, Wk, bk_, Wv, bv_, Wo, bo_, seq=S):
    """Shard full inputs into per-core input maps."""
    ident = np.eye(128, dtype=np.float32)
    in_maps = []
    for c in range(NCORES):
        b, g = c // GROUP, c % GROUP
        sl = slice(DPC * g, DPC * (g + 1))
        in_maps.append({
            "xq": np.ascontiguousarray(query[b, :seq]),
            "xk": np.ascontiguousarray(key[b, :seq]),
            "xv": np.ascontiguousarray(value[b, :seq]),
            "wqT": np.ascontiguousarray(Wq[sl, :].T),
            "wkT": np.ascontiguousarray(Wk[sl, :].T),
            "wvT": np.ascontiguousarray(Wv[sl, :].T),
            "woT": np.ascontiguousarray(Wo[:, sl].T),
            "bq": np.ascontiguousarray(bq_[sl].reshape(DPC, 1)),
            "bk": np.ascontiguousarray(bk_[sl].reshape(DPC, 1)),
            "bv": np.ascontiguousarray(bv_[sl].reshape(1, DPC)),
            "bo": np.ascontiguousarray(bo_.reshape(1, D)),
            "ident": ident,
        })
    return in_maps


def assemble(results, seq=S):
    NCHUNK = seq // 512
    out = np.empty((B, seq, D), dtype=np.float32)
    for core in range(NCORES):
        b, g = core // GROUP, core % GROUP
        r = results[core]["out"]
        for c in range(NCHUNK):
            out[b, 512 * c + 128 * g:512 * c + 128 * (g + 1), :] = \
                r[128 * c:128 * (c + 1), :]
    return out


_COMPILED = None


def kernel(query, key, value, Wq, bq, Wk, bk, Wv, bv, Wo, bo):
    global _COMPILED
    if _COMPILED is None:
        _COMPILED = build()
    in_maps = make_in_maps(np.asarray(query, np.float32), np.asarray(key, np.float32),
                           np.asarray(value, np.float32), np.asarray(Wq, np.float32),
                           np.asarray(bq, np.float32), np.asarray(Wk, np.float32),
                           np.asarray(bk, np.float32), np.asarray(Wv, np.float32),
                           np.asarray(bv, np.float32), np.asarray(Wo, np.float32),
                           np.asarray(bo, np.float32))
    res = run_bass_kernel_spmd(_COMPILED, in_maps, list(range(NCORES)))
    return assemble(res.results)

